# revision 11
# baseline (speedup 1.0000x reference)
"""ConvTreeGRUCell on 8 Trainium2 NeuronCores.

Sharding: spatial over H. Each core owns 24 output rows (192/8) and
receives a 28-row input slab (2-row halo each side, zero-padded at the
image borders on the host). All convs and L-reductions are local.

v2 (vs v1 bf16 restructure):
  - reset-gate convs (P0 x-part and the per-child-pair taps) run in
    fp8e4m3 DoubleRow mode: taps are paired into [K, 2, N] access
    patterns (second k-tile = the partner tap at column delta), so each
    DR matmul does 2 taps at 0.5 cycles/row.  Weights are scaled x32 on
    the host (keeps them out of the fp8 subnormal range); the
    activation un-scales with scale=1/32.  The xr2 inject uses a 32*I
    bf16 identity so the whole PSUM is uniformly scaled.
  - child_sum is accumulated exactly on the PE ([I;I] x cat_p, 4
    accumulating bf16 matmuls) and kept in f32 for the h-combine path
    (csum32); only the z-conv rhs copy is bf16.  This halves the
    elementwise error vs the bf16 sum tree.
  - z/o convs stay bf16 (fp8 on the big-magnitude csum input blows the
    error budget ~6e-2).
  - reset_hidden products accumulate window-wise into T (bf16) on
    Vector; cross-partition folds stay [I;I] matmuls.
"""

import os
import sys

import numpy as np
import ml_dtypes

for _p in ("/opt/trn_rl_repo",):
    if _p not in sys.path and os.path.isdir(_p):
        sys.path.insert(0, _p)

import concourse.bass as bass
import concourse.tile as tile
from concourse import bacc
from concourse import mybir
from concourse.ap import AP
from concourse.bass_utils import run_bass_kernel_spmd

F32 = mybir.dt.float32
BF16 = mybir.dt.bfloat16
FP8 = mybir.dt.float8e4
NPBF16 = ml_dtypes.bfloat16
NPFP8 = mybir.dt.np(FP8)
DR = mybir.MatmulPerfMode.DoubleRow
WSCALE = 32.0

C = 64          # channels
L = 8           # children
HW = 192        # image H and W
NCORES = 8
OUT_ROWS = HW // NCORES          # 24 output rows per core
IN_ROWS = OUT_ROWS + 4           # 28-row slab (2-row halo each side)
WP = HW + 2                      # 194: padded row width
FRAME = IN_ROWS * WP             # 5432
FREE = FRAME + 2                 # 5434: +1 front pad, +1 tail pad

# flat index of (row r, col c) in the frame = 1 + r*WP + c
S1_LO = 1 + 1 * WP               # 195   (r rows 1..26)
S1_HI = 1 + 26 * WP + 194        # 5239 (exclusive)
S1N = S1_HI - S1_LO              # 5044
S2_LO = 1 + 2 * WP               # 389   (h rows 2..25)
S2_HI = 1 + 25 * WP + 194        # 5045 (exclusive)
S2N = S2_HI - S2_LO              # 4656

NWIN = 512

TAP_OFF = [dy * WP + dx for dy in (-1, 0, 1) for dx in (-1, 0, 1)]
# DoubleRow tap pairs: (0,1) (2,3) (4,5) (6,7) + (8, partner)
# P0: partner = tap8 again with zero weights.  Pairs: partner = the xr8
# inject region living at cat8x cols [FREE, FREE+S1N) -> constant delta
# FREE - S1_LO - TAP_OFF[8] from the tap-8 window base.
DR_PAIRS = [(0, 1), (2, 3), (4, 5), (6, 7), (8, 8)]
XR_COL = None  # set below once constants exist


def _windows(lo, hi):
    out = []
    s = lo
    while s < hi:
        out.append((s, min(NWIN, hi - s)))
        s += NWIN
    return out


S1WIN = _windows(S1_LO, S1_HI)
S2WIN = _windows(S2_LO, S2_HI)

_BUILT = None


def _dr_rhs(tile_ap, base_col, n, delta):
    """[K, 2, N] moving AP: k-tile 0 at base_col, k-tile 1 at +delta."""
    sl = tile_ap[:, base_col:base_col + n]
    dims = [list(d) for d in sl.ap]
    assert len(dims) == 2
    return AP(sl.tensor, sl.offset, [dims[0], [delta, 2], [1, n]])


def build_program():
    nc = bacc.Bacc("TRN2")

    x8t = nc.dram_tensor("x8t", [C, FREE], FP8, kind="ExternalInput")
    xin = nc.dram_tensor("xin", [C, FREE], BF16, kind="ExternalInput")
    cin8 = nc.dram_tensor("cin8", [L, C, FREE], FP8, kind="ExternalInput")
    cin = nc.dram_tensor("cin", [L, C, FREE], BF16, kind="ExternalInput")
    wrxt = nc.dram_tensor("wrxt", [2 * C, 5, 2, 2 * C], FP8, kind="ExternalInput")
    wrct = nc.dram_tensor("wrct", [2 * C, 5, 2, 2 * C], FP8, kind="ExternalInput")
    wzt = nc.dram_tensor("wzt", [2 * C, 9, C], BF16, kind="ExternalInput")
    wot = nc.dram_tensor("wot", [2 * C, 9, C], BF16, kind="ExternalInput")
    idvt = nc.dram_tensor("idvt", [2 * C, C], BF16, kind="ExternalInput")
    brt = nc.dram_tensor("brt", [2 * C, 1], F32, kind="ExternalInput")
    bzt = nc.dram_tensor("bzt", [C, 1], F32, kind="ExternalInput")
    bot = nc.dram_tensor("bot", [C, 1], F32, kind="ExternalInput")
    hout = nc.dram_tensor("hout", [C, S2N], BF16, kind="ExternalOutput")

    ID = mybir.ActivationFunctionType.Identity
    SIG = mybir.ActivationFunctionType.Sigmoid
    TANH = mybir.ActivationFunctionType.Tanh
    CP = mybir.ActivationFunctionType.Copy
    INV = 1.0 / WSCALE

    with tile.TileContext(nc) as tc:
        with (
            tc.tile_pool(name="singles", bufs=1) as singles,
            tc.tile_pool(name="cats", bufs=1) as cats,
            tc.tile_pool(name="rbp", bufs=3) as rb_pool,
            tc.tile_pool(name="hwp", bufs=3) as hw_pool,
            tc.tile_pool(name="psum", bufs=4, space="PSUM") as psum_pool,
            tc.tile_pool(name="psumf", bufs=4, space="PSUM") as psumf_pool,
        ):
            # ---- persistent tiles ----
            x8 = singles.tile([2 * C, FREE], FP8, tag="x8")
            wrx = singles.tile([2 * C, 5, 2, 2 * C], FP8, tag="wrx")
            wrc = singles.tile([2 * C, 5, 2, 2 * C], FP8, tag="wrc")
            wz = singles.tile([2 * C, 9, C], BF16, tag="wz")
            wo = singles.tile([2 * C, 9, C], BF16, tag="wo")
            i2v = singles.tile([2 * C, C], BF16, tag="i2v")       # [I;I]
            br = singles.tile([2 * C, 1], F32, tag="br")
            bz = singles.tile([C, 1], F32, tag="bz")
            bo = singles.tile([C, 1], F32, tag="bo")
            zs = singles.tile([2 * C, FREE], BF16, tag="zs")      # [csum | x]
            orh = singles.tile([2 * C, FREE], BF16, tag="orh")    # [rh | x]
            csum32 = singles.tile([C, S2N], F32, tag="csum32")
            T = singles.tile([2 * C, S1N], BF16, tag="T")         # sum r*child
            zb = singles.tile([C, S2N], BF16, tag="zb")
            ob = singles.tile([C, S2N], BF16, tag="ob")

            # ---- loads: matmul-critical order ----
            nc.sync.dma_start(out=x8[0:C, :], in_=x8t[:])
            nc.sync.dma_start(out=x8[C:2 * C, :], in_=x8t[:])
            nc.sync.dma_start(out=wrx, in_=wrxt[:])
            nc.sync.dma_start(out=br, in_=brt[:])
            nc.sync.dma_start(out=wrc, in_=wrct[:])
            cat8t, catt = [], []
            for p in range(4):
                c8 = cats.tile([2 * C, FREE + S1N], FP8, tag=f"cat8_{p}")
                nc.sync.dma_start(out=c8[0:C, 0:FREE], in_=cin8[2 * p])
                nc.sync.dma_start(out=c8[C:2 * C, 0:FREE], in_=cin8[2 * p + 1])
                cat8t.append(c8)
                cb = cats.tile([2 * C, FREE], BF16, tag=f"cat{p}")
                nc.sync.dma_start(out=cb[0:C, :], in_=cin[2 * p])
                nc.sync.dma_start(out=cb[C:2 * C, :], in_=cin[2 * p + 1])
                catt.append(cb)
            nc.sync.dma_start(out=i2v, in_=idvt[:])
            nc.sync.dma_start(out=wz, in_=wzt[:])
            nc.sync.dma_start(out=wo, in_=wot[:])
            nc.sync.dma_start(out=bz, in_=bzt[:])
            nc.sync.dma_start(out=bo, in_=bot[:])
            nc.sync.dma_start(out=zs[C:2 * C, :], in_=xin[:])
            nc.sync.dma_start(out=orh[C:2 * C, :], in_=xin[:])
            # zero the csum/rh halves (pad cols outside S1 must be 0)
            nc.scalar.memzero(zs[0:C, :])
            nc.scalar.memzero(orh[0:C, :])

            # ---- P0: xr2 = [Wr_x*x + br] (x2 on halves), fp8 DoubleRow ----
            for s, n in S1WIN:
                j = s - S1_LO
                ps = psum_pool.tile([2 * C, NWIN], F32, tag="ps")
                for i, (ta, tb) in enumerate(DR_PAIRS):
                    oa = TAP_OFF[ta]
                    nc.tensor.matmul(
                        out=ps[:, :n],
                        lhsT=wrx[:, i, :, :],
                        rhs=_dr_rhs(x8, s + oa, n, TAP_OFF[tb] - oa),
                        start=(i == 0),
                        stop=(i == 4),
                        perf_mode=DR,
                    )
                for p in range(4):
                    nc.scalar.activation(
                        out=cat8t[p][:, FREE + j:FREE + j + n], in_=ps[:, :n],
                        func=ID, bias=br[:, 0:1], scale=INV,
                    )

            # ---- stage 1: children pairs (fp8 DR taps + bf16 inject) ----
            for p in range(4):
                c8 = cat8t[p]
                cb = catt[p]
                for s, n in S1WIN:
                    j = s - S1_LO
                    ps = psum_pool.tile([2 * C, NWIN], F32, tag="ps")
                    for i, (ta, tb) in enumerate(DR_PAIRS):
                        oa = TAP_OFF[ta]
                        if i < 4:
                            delta = TAP_OFF[tb] - oa
                        else:  # pair (tap8, xr-inject region)
                            delta = FREE - S1_LO - oa
                        nc.tensor.matmul(
                            out=ps[:, :n],
                            lhsT=wrc[:, i, :, :],
                            rhs=_dr_rhs(c8, s + oa, n, delta),
                            start=(i == 0),
                            stop=(i == 4),
                            perf_mode=DR,
                        )
                    rb = rb_pool.tile([2 * C, NWIN], BF16, tag="rb")
                    nc.scalar.activation(
                        out=rb[:, :n], in_=ps[:, :n], func=SIG, scale=INV,
                    )
                    # T[:, w] (+)= rb * child  (bf16, 128 partitions)
                    if p == 0:
                        nc.vector.tensor_mul(
                            out=T[:, j:j + n], in0=rb[:, :n], in1=cb[:, s:s + n]
                        )
                    else:
                        tm = rb_pool.tile([2 * C, NWIN], BF16, tag="tm")
                        nc.vector.tensor_mul(
                            out=tm[:, :n], in0=rb[:, :n], in1=cb[:, s:s + n]
                        )
                        nc.vector.tensor_add(
                            out=T[:, j:j + n], in0=T[:, j:j + n], in1=tm[:, :n]
                        )

            # ---- csum: PE-accumulated exact sum of all 8 children ----
            for s, n in S1WIN:
                ps = psumf_pool.tile([C, NWIN], F32, tag="psf")
                for p in range(4):
                    nc.tensor.matmul(
                        out=ps[:, :n], lhsT=i2v, rhs=catt[p][:, s:s + n],
                        start=(p == 0), stop=(p == 3),
                    )
                nc.scalar.activation(out=zs[0:C, s:s + n], in_=ps[:, :n], func=CP)
                ov_lo, ov_hi = max(s, S2_LO), min(s + n, S2_HI)
                if ov_lo < ov_hi:
                    nc.vector.tensor_copy(
                        out=csum32[:, ov_lo - S2_LO:ov_hi - S2_LO],
                        in_=ps[:, ov_lo - s:ov_hi - s],
                    )

            # ---- z conv (bf16) ----
            for s, n in S2WIN:
                j = s - S2_LO
                ps = psumf_pool.tile([C, NWIN], F32, tag="psf")
                for t in range(9):
                    o = TAP_OFF[t]
                    nc.tensor.matmul(
                        out=ps[:, :n],
                        lhsT=wz[:, t, :],
                        rhs=zs[:, s + o:s + o + n],
                        start=(t == 0),
                        stop=(t == 8),
                    )
                nc.scalar.activation(
                    out=zb[:, j:j + n], in_=ps[:, :n], func=SIG, bias=bz[:, 0:1]
                )

            # ---- rh fold: orh[0:C] = T_low + T_high ----
            for s, n in S1WIN:
                j = s - S1_LO
                ps = psumf_pool.tile([C, NWIN], F32, tag="psf")
                nc.tensor.matmul(out=ps[:, :n], lhsT=i2v, rhs=T[:, j:j + n])
                nc.vector.tensor_copy(out=orh[0:C, s:s + n], in_=ps[:, :n])

            # ---- o conv + h combine + store, per window ----
            for s, n in S2WIN:
                j = s - S2_LO
                ps = psumf_pool.tile([C, NWIN], F32, tag="psf")
                for t in range(9):
                    o = TAP_OFF[t]
                    nc.tensor.matmul(
                        out=ps[:, :n],
                        lhsT=wo[:, t, :],
                        rhs=orh[:, s + o:s + o + n],
                        start=(t == 0),
                        stop=(t == 8),
                    )
                nc.scalar.activation(
                    out=ob[:, j:j + n], in_=ps[:, :n], func=TANH, bias=bo[:, 0:1]
                )
                # h = o + z*(csum - o), csum path in f32
                t1 = hw_pool.tile([C, NWIN], F32, tag="t1")
                nc.vector.scalar_tensor_tensor(
                    out=t1[:, :n],
                    in0=ob[:, j:j + n],
                    scalar=-1.0,
                    in1=csum32[:, j:j + n],
                    op0=mybir.AluOpType.mult,
                    op1=mybir.AluOpType.add,
                )
                nc.vector.tensor_mul(out=t1[:, :n], in0=zb[:, j:j + n], in1=t1[:, :n])
                hst = hw_pool.tile([C, NWIN], BF16, tag="hst")
                nc.vector.tensor_add(out=hst[:, :n], in0=ob[:, j:j + n], in1=t1[:, :n])
                nc.sync.dma_start(out=hout[:, j:j + n], in_=hst[:, :n])

    nc.finalize()
    return nc


def _get_program():
    global _BUILT
    if _BUILT is None:
        _BUILT = build_program()
    return _BUILT


def make_in_maps(x, child_h, Wr, br, Wz, bz, Wo, bo):
    """Host-side sharding: pad borders/columns, slice 28-row slabs."""
    x = np.asarray(x, dtype=np.float32)
    child_h = np.asarray(child_h, dtype=np.float32)

    xp = np.zeros((C, HW + 4, WP), dtype=np.float32)
    xp[:, 2:2 + HW, 1:1 + HW] = x[0]
    cp = np.zeros((L, C, HW + 4, WP), dtype=np.float32)
    cp[:, :, 2:2 + HW, 1:1 + HW] = child_h[:, 0]

    def frame(a, dt):  # [..., IN_ROWS, WP] -> [..., FREE] with front/tail pad
        flat = a.reshape(a.shape[:-2] + (FRAME,))
        out = np.zeros(a.shape[:-2] + (FREE,), dtype=dt)
        out[..., 1:1 + FRAME] = flat.astype(dt)
        return out

    def wt(w):  # [C, 2C, 3, 3] -> [2C(in), 9, C(out)]; in 0:C = x-half
        return np.transpose(np.asarray(w, np.float32), (1, 2, 3, 0)).reshape(2 * C, 9, C)

    def drpack(w64, rows):
        """w64: [C(in), 9, C(out)] x-or-child half -> [2C, 5, 2, 2C] fp8 x32.
        rows: (row offset pairs) describing where the in-channels sit for
        each output half; here we place per spec below."""
        out = np.zeros((2 * C, 5, 2, 2 * C), dtype=np.float32)
        for i, (ta, tb) in enumerate(DR_PAIRS):
            for k, tap in ((0, ta), (1, tb)):
                if i == 4 and k == 1:
                    continue  # second k-tile of pair 4 handled by caller
                for (rlo, clo) in rows:
                    out[rlo:rlo + C, i, k, clo:clo + C] = w64[:, tap, :]
        return out

    wrt = wt(Wr)
    # P0: x channels on partitions 0:C (and a copy of x on C:2C that gets
    # zero weights); outputs [xr | xr] -> weight blocks (0,0) and (0,C)
    wrx = (drpack(wrt[0:C], [(0, 0), (0, C)]) * WSCALE).astype(NPFP8)
    # pairs: block-diag child weights; pair-4 k-tile 1 = identity (xr inject)
    wrcf = drpack(wrt[C:2 * C], [(0, 0), (C, C)]) * WSCALE
    wrcf[:, 4, 1, :] = WSCALE * np.eye(2 * C)
    wrc = wrcf.astype(NPFP8)

    def wswap(w):  # z/o lhsT with [csum/rh | x] partition order
        a = wt(w)
        return np.ascontiguousarray(
            np.concatenate([a[C:2 * C], a[0:C]], axis=0)
        ).astype(NPBF16)

    wzt, wot = wswap(Wz), wswap(Wo)
    idvt = np.concatenate([np.eye(C), np.eye(C)], axis=0).astype(NPBF16)
    brt = np.tile(np.asarray(br, np.float32).reshape(C, 1), (2, 1))
    bzt = np.asarray(bz, np.float32).reshape(C, 1)
    bot = np.asarray(bo, np.float32).reshape(C, 1)

    in_maps = []
    for k in range(NCORES):
        r0 = k * OUT_ROWS  # slab = global rows r0-2 .. r0+26
        xs = xp[:, r0:r0 + IN_ROWS, :]
        cs = cp[:, :, r0:r0 + IN_ROWS, :]
        in_maps.append({
            "x8t": frame(xs, NPFP8), "xin": frame(xs, NPBF16),
            "cin8": frame(cs, NPFP8), "cin": frame(cs, NPBF16),
            "wrxt": wrx, "wrct": wrc, "wzt": wzt, "wot": wot,
            "idvt": idvt,
            "brt": brt, "bzt": bzt, "bot": bot,
        })
    return in_maps


def run(in_maps, trace=False):
    nc = _get_program()
    return run_bass_kernel_spmd(nc, in_maps, list(range(NCORES)), trace=trace)


def kernel(x, child_h, Wr, br, Wz, bz, Wo, bo):
    in_maps = make_in_maps(x, child_h, Wr, br, Wz, bz, Wo, bo)
    res = run(in_maps).results
    out = np.empty((1, C, HW, HW), dtype=np.float32)
    for k in range(NCORES):
        h = np.asarray(res[k]["hout"]).astype(np.float32)
        h = h.reshape(C, OUT_ROWS, WP)[:, :, 1:1 + HW]
        out[0, :, k * OUT_ROWS:(k + 1) * OUT_ROWS, :] = h
    return out


# revision 12
# speedup vs baseline: 1.0762x; 1.0762x over previous
"""ConvTreeGRUCell on 8 Trainium2 NeuronCores.

Sharding: spatial over H. Each core owns 24 output rows (192/8) and
receives a 28-row input slab (2-row halo each side, zero-padded at the
image borders on the host). All convs and L-reductions are local.

v2 (vs v1 bf16 restructure):
  - reset-gate convs (P0 x-part and the per-child-pair taps) run in
    fp8e4m3 DoubleRow mode: taps are paired into [K, 2, N] access
    patterns (second k-tile = the partner tap at column delta), so each
    DR matmul does 2 taps at 0.5 cycles/row.  Weights are scaled x32 on
    the host (keeps them out of the fp8 subnormal range); the
    activation un-scales with scale=1/32.  The xr2 inject uses a 32*I
    bf16 identity so the whole PSUM is uniformly scaled.
  - child_sum is accumulated exactly on the PE ([I;I] x cat_p, 4
    accumulating bf16 matmuls) and kept in f32 for the h-combine path
    (csum32); only the z-conv rhs copy is bf16.  This halves the
    elementwise error vs the bf16 sum tree.
  - z/o convs stay bf16 (fp8 on the big-magnitude csum input blows the
    error budget ~6e-2).
  - reset_hidden products accumulate window-wise into T (bf16) on
    Vector; cross-partition folds stay [I;I] matmuls.
"""

import os
import sys

import numpy as np
import ml_dtypes

for _p in ("/opt/trn_rl_repo",):
    if _p not in sys.path and os.path.isdir(_p):
        sys.path.insert(0, _p)

import concourse.bass as bass
import concourse.tile as tile
from concourse import bacc
from concourse import mybir
from concourse.ap import AP
from concourse.bass_utils import run_bass_kernel_spmd

F32 = mybir.dt.float32
BF16 = mybir.dt.bfloat16
FP8 = mybir.dt.float8e4
NPBF16 = ml_dtypes.bfloat16
NPFP8 = mybir.dt.np(FP8)
DR = mybir.MatmulPerfMode.DoubleRow
WSCALE = 32.0

C = 64          # channels
L = 8           # children
HW = 192        # image H and W
NCORES = 8
OUT_ROWS = HW // NCORES          # 24 output rows per core
IN_ROWS = OUT_ROWS + 4           # 28-row slab (2-row halo each side)
WP = HW + 2                      # 194: padded row width
FRAME = IN_ROWS * WP             # 5432
FREE = FRAME + 2                 # 5434: +1 front pad, +1 tail pad

# flat index of (row r, col c) in the frame = 1 + r*WP + c
S1_LO = 1 + 1 * WP               # 195   (r rows 1..26)
S1_HI = 1 + 26 * WP + 194        # 5239 (exclusive)
S1N = S1_HI - S1_LO              # 5044
S2_LO = 1 + 2 * WP               # 389   (h rows 2..25)
S2_HI = 1 + 25 * WP + 194        # 5045 (exclusive)
S2N = S2_HI - S2_LO              # 4656

NWIN = 512

TAP_OFF = [dy * WP + dx for dy in (-1, 0, 1) for dx in (-1, 0, 1)]
# DoubleRow tap pairs: (0,1) (2,3) (4,5) (6,7) + (8, partner)
# P0: partner = tap8 again with zero weights.  Pairs: partner = the xr8
# inject region living at cat8x cols [FREE, FREE+S1N) -> constant delta
# FREE - S1_LO - TAP_OFF[8] from the tap-8 window base.
DR_PAIRS = [(0, 1), (2, 3), (4, 5), (6, 7), (8, 8)]
XR_COL = None  # set below once constants exist


def _windows(lo, hi):
    out = []
    s = lo
    while s < hi:
        out.append((s, min(NWIN, hi - s)))
        s += NWIN
    return out


S1WIN = _windows(S1_LO, S1_HI)
S2WIN = _windows(S2_LO, S2_HI)

_BUILT = None


def _dr_rhs(tile_ap, base_col, n, delta):
    """[K, 2, N] moving AP: k-tile 0 at base_col, k-tile 1 at +delta."""
    sl = tile_ap[:, base_col:base_col + n]
    dims = [list(d) for d in sl.ap]
    assert len(dims) == 2
    return AP(sl.tensor, sl.offset, [dims[0], [delta, 2], [1, n]])


def build_program():
    nc = bacc.Bacc("TRN2")

    x8t = nc.dram_tensor("x8t", [C, FREE], FP8, kind="ExternalInput")
    xin = nc.dram_tensor("xin", [C, FREE], BF16, kind="ExternalInput")
    cin8 = nc.dram_tensor("cin8", [L, C, FREE], FP8, kind="ExternalInput")
    cin = nc.dram_tensor("cin", [L, C, FREE], BF16, kind="ExternalInput")
    wrxt = nc.dram_tensor("wrxt", [2 * C, 5, 2, 2 * C], FP8, kind="ExternalInput")
    wrct = nc.dram_tensor("wrct", [2 * C, 5, 2, 2 * C], FP8, kind="ExternalInput")
    wzt = nc.dram_tensor("wzt", [2 * C, 9, C], BF16, kind="ExternalInput")
    wot = nc.dram_tensor("wot", [2 * C, 9, C], BF16, kind="ExternalInput")
    idvt = nc.dram_tensor("idvt", [2 * C, C], BF16, kind="ExternalInput")
    brt = nc.dram_tensor("brt", [2 * C, 1], F32, kind="ExternalInput")
    bzt = nc.dram_tensor("bzt", [C, 1], F32, kind="ExternalInput")
    bot = nc.dram_tensor("bot", [C, 1], F32, kind="ExternalInput")
    hout = nc.dram_tensor("hout", [C, S2N], BF16, kind="ExternalOutput")

    ID = mybir.ActivationFunctionType.Identity
    SIG = mybir.ActivationFunctionType.Sigmoid
    TANH = mybir.ActivationFunctionType.Tanh
    CP = mybir.ActivationFunctionType.Copy
    INV = 1.0 / WSCALE

    with tile.TileContext(nc) as tc:
        with (
            tc.tile_pool(name="singles", bufs=1) as singles,
            tc.tile_pool(name="cats", bufs=1) as cats,
            tc.tile_pool(name="rbp", bufs=3) as rb_pool,
            tc.tile_pool(name="hwp", bufs=3) as hw_pool,
            tc.tile_pool(name="psum", bufs=4, space="PSUM") as psum_pool,
            tc.tile_pool(name="psumf", bufs=4, space="PSUM") as psumf_pool,
        ):
            # ---- persistent tiles ----
            x8 = singles.tile([2 * C, FREE], FP8, tag="x8")
            wrx = singles.tile([2 * C, 5, 2, 2 * C], FP8, tag="wrx")
            wrc = singles.tile([2 * C, 5, 2, 2 * C], FP8, tag="wrc")
            wz = singles.tile([2 * C, 9, C], BF16, tag="wz")
            wo = singles.tile([2 * C, 9, C], BF16, tag="wo")
            i2v = singles.tile([2 * C, C], BF16, tag="i2v")       # [I;I]
            br = singles.tile([2 * C, 1], F32, tag="br")
            bz = singles.tile([C, 1], F32, tag="bz")
            bo = singles.tile([C, 1], F32, tag="bo")
            zs = singles.tile([2 * C, FREE], BF16, tag="zs")      # [csum | x]
            orh = singles.tile([2 * C, FREE], BF16, tag="orh")    # [rh | x]
            csum32 = singles.tile([C, S2N], F32, tag="csum32")
            T = singles.tile([2 * C, S1N], BF16, tag="T")         # sum r*child
            zb = singles.tile([C, S2N], BF16, tag="zb")
            ob = singles.tile([C, S2N], BF16, tag="ob")

            # ---- loads: matmul-critical order ----
            nc.sync.dma_start(out=x8[0:C, :], in_=x8t[:])
            nc.sync.dma_start(out=x8[C:2 * C, :], in_=x8t[:])
            nc.sync.dma_start(out=wrx, in_=wrxt[:])
            nc.sync.dma_start(out=br, in_=brt[:])
            nc.sync.dma_start(out=wrc, in_=wrct[:])
            cat8t, catt = [], []
            for p in range(4):
                c8 = cats.tile([2 * C, FREE + S1N], FP8, tag=f"cat8_{p}")
                nc.sync.dma_start(out=c8[0:C, 0:FREE], in_=cin8[2 * p])
                nc.sync.dma_start(out=c8[C:2 * C, 0:FREE], in_=cin8[2 * p + 1])
                cat8t.append(c8)
                cb = cats.tile([2 * C, FREE], BF16, tag=f"cat{p}")
                nc.sync.dma_start(out=cb[0:C, :], in_=cin[2 * p])
                nc.sync.dma_start(out=cb[C:2 * C, :], in_=cin[2 * p + 1])
                catt.append(cb)
            nc.sync.dma_start(out=i2v, in_=idvt[:])
            nc.sync.dma_start(out=wz, in_=wzt[:])
            nc.sync.dma_start(out=wo, in_=wot[:])
            nc.sync.dma_start(out=bz, in_=bzt[:])
            nc.sync.dma_start(out=bo, in_=bot[:])
            nc.sync.dma_start(out=zs[C:2 * C, :], in_=xin[:])
            nc.sync.dma_start(out=orh[C:2 * C, :], in_=xin[:])
            # zero the csum/rh halves (pad cols outside S1 must be 0)
            nc.scalar.memzero(zs[0:C, :])
            nc.scalar.memzero(orh[0:C, :])

            # ---- P0: xr2 = [Wr_x*x + br] (x2 on halves), fp8 DoubleRow ----
            for s, n in S1WIN:
                j = s - S1_LO
                ps = psum_pool.tile([2 * C, NWIN], F32, tag="ps")
                for i, (ta, tb) in enumerate(DR_PAIRS):
                    oa = TAP_OFF[ta]
                    nc.tensor.matmul(
                        out=ps[:, :n],
                        lhsT=wrx[:, i, :, :],
                        rhs=_dr_rhs(x8, s + oa, n, TAP_OFF[tb] - oa),
                        start=(i == 0),
                        stop=(i == 4),
                        perf_mode=DR,
                    )
                nc.scalar.activation(
                    out=cat8t[0][:, FREE + j:FREE + j + n], in_=ps[:, :n],
                    func=ID, bias=br[:, 0:1], scale=INV,
                )
                for p in range(1, 4):
                    nc.vector.tensor_copy(
                        out=cat8t[p][:, FREE + j:FREE + j + n],
                        in_=cat8t[0][:, FREE + j:FREE + j + n],
                    )

            # ---- stage 1: children pairs (fp8 DR taps + bf16 inject) ----
            for p in range(4):
                c8 = cat8t[p]
                cb = catt[p]
                for s, n in S1WIN:
                    j = s - S1_LO
                    ps = psum_pool.tile([2 * C, NWIN], F32, tag="ps")
                    for i, (ta, tb) in enumerate(DR_PAIRS):
                        oa = TAP_OFF[ta]
                        if i < 4:
                            delta = TAP_OFF[tb] - oa
                        else:  # pair (tap8, xr-inject region)
                            delta = FREE - S1_LO - oa
                        nc.tensor.matmul(
                            out=ps[:, :n],
                            lhsT=wrc[:, i, :, :],
                            rhs=_dr_rhs(c8, s + oa, n, delta),
                            start=(i == 0),
                            stop=(i == 4),
                            perf_mode=DR,
                        )
                    rb = rb_pool.tile([2 * C, NWIN], BF16, tag="rb")
                    nc.scalar.activation(
                        out=rb[:, :n], in_=ps[:, :n], func=SIG, scale=INV,
                    )
                    # T[:, w] (+)= rb * child  (bf16, 128 partitions)
                    if p == 0:
                        nc.vector.tensor_mul(
                            out=T[:, j:j + n], in0=rb[:, :n], in1=cb[:, s:s + n]
                        )
                    else:
                        tm = rb_pool.tile([2 * C, NWIN], BF16, tag="tm")
                        nc.vector.tensor_mul(
                            out=tm[:, :n], in0=rb[:, :n], in1=cb[:, s:s + n]
                        )
                        nc.vector.tensor_add(
                            out=T[:, j:j + n], in0=T[:, j:j + n], in1=tm[:, :n]
                        )

            # ---- csum: PE-accumulated exact sum of all 8 children ----
            for s, n in S1WIN:
                ps = psumf_pool.tile([C, NWIN], F32, tag="psf")
                for p in range(4):
                    nc.tensor.matmul(
                        out=ps[:, :n], lhsT=i2v, rhs=catt[p][:, s:s + n],
                        start=(p == 0), stop=(p == 3),
                    )
                nc.scalar.activation(out=zs[0:C, s:s + n], in_=ps[:, :n], func=CP)
                ov_lo, ov_hi = max(s, S2_LO), min(s + n, S2_HI)
                if ov_lo < ov_hi:
                    nc.vector.tensor_copy(
                        out=csum32[:, ov_lo - S2_LO:ov_hi - S2_LO],
                        in_=ps[:, ov_lo - s:ov_hi - s],
                    )

            # ---- z conv (bf16) ----
            for s, n in S2WIN:
                j = s - S2_LO
                ps = psumf_pool.tile([C, NWIN], F32, tag="psf")
                for t in range(9):
                    o = TAP_OFF[t]
                    nc.tensor.matmul(
                        out=ps[:, :n],
                        lhsT=wz[:, t, :],
                        rhs=zs[:, s + o:s + o + n],
                        start=(t == 0),
                        stop=(t == 8),
                    )
                nc.scalar.activation(
                    out=zb[:, j:j + n], in_=ps[:, :n], func=SIG, bias=bz[:, 0:1]
                )

            # ---- rh fold: orh[0:C] = T_low + T_high ----
            for s, n in S1WIN:
                j = s - S1_LO
                ps = psumf_pool.tile([C, NWIN], F32, tag="psf")
                nc.tensor.matmul(out=ps[:, :n], lhsT=i2v, rhs=T[:, j:j + n])
                nc.vector.tensor_copy(out=orh[0:C, s:s + n], in_=ps[:, :n])

            # ---- o conv + h combine + store, per window ----
            for s, n in S2WIN:
                j = s - S2_LO
                ps = psumf_pool.tile([C, NWIN], F32, tag="psf")
                for t in range(9):
                    o = TAP_OFF[t]
                    nc.tensor.matmul(
                        out=ps[:, :n],
                        lhsT=wo[:, t, :],
                        rhs=orh[:, s + o:s + o + n],
                        start=(t == 0),
                        stop=(t == 8),
                    )
                nc.scalar.activation(
                    out=ob[:, j:j + n], in_=ps[:, :n], func=TANH, bias=bo[:, 0:1]
                )
                # h = o + z*(csum - o), csum path in f32
                t1 = hw_pool.tile([C, NWIN], F32, tag="t1")
                nc.vector.scalar_tensor_tensor(
                    out=t1[:, :n],
                    in0=ob[:, j:j + n],
                    scalar=-1.0,
                    in1=csum32[:, j:j + n],
                    op0=mybir.AluOpType.mult,
                    op1=mybir.AluOpType.add,
                )
                nc.vector.tensor_mul(out=t1[:, :n], in0=zb[:, j:j + n], in1=t1[:, :n])
                hst = hw_pool.tile([C, NWIN], BF16, tag="hst")
                nc.vector.tensor_add(out=hst[:, :n], in0=ob[:, j:j + n], in1=t1[:, :n])
                nc.sync.dma_start(out=hout[:, j:j + n], in_=hst[:, :n])

    nc.finalize()
    return nc


def _get_program():
    global _BUILT
    if _BUILT is None:
        _BUILT = build_program()
    return _BUILT


def make_in_maps(x, child_h, Wr, br, Wz, bz, Wo, bo):
    """Host-side sharding: pad borders/columns, slice 28-row slabs."""
    x = np.asarray(x, dtype=np.float32)
    child_h = np.asarray(child_h, dtype=np.float32)

    xp = np.zeros((C, HW + 4, WP), dtype=np.float32)
    xp[:, 2:2 + HW, 1:1 + HW] = x[0]
    cp = np.zeros((L, C, HW + 4, WP), dtype=np.float32)
    cp[:, :, 2:2 + HW, 1:1 + HW] = child_h[:, 0]

    def frame(a, dt):  # [..., IN_ROWS, WP] -> [..., FREE] with front/tail pad
        flat = a.reshape(a.shape[:-2] + (FRAME,))
        out = np.zeros(a.shape[:-2] + (FREE,), dtype=dt)
        out[..., 1:1 + FRAME] = flat.astype(dt)
        return out

    def wt(w):  # [C, 2C, 3, 3] -> [2C(in), 9, C(out)]; in 0:C = x-half
        return np.transpose(np.asarray(w, np.float32), (1, 2, 3, 0)).reshape(2 * C, 9, C)

    def drpack(w64, rows):
        """w64: [C(in), 9, C(out)] x-or-child half -> [2C, 5, 2, 2C] fp8 x32.
        rows: (row offset pairs) describing where the in-channels sit for
        each output half; here we place per spec below."""
        out = np.zeros((2 * C, 5, 2, 2 * C), dtype=np.float32)
        for i, (ta, tb) in enumerate(DR_PAIRS):
            for k, tap in ((0, ta), (1, tb)):
                if i == 4 and k == 1:
                    continue  # second k-tile of pair 4 handled by caller
                for (rlo, clo) in rows:
                    out[rlo:rlo + C, i, k, clo:clo + C] = w64[:, tap, :]
        return out

    wrt = wt(Wr)
    # P0: x channels on partitions 0:C (and a copy of x on C:2C that gets
    # zero weights); outputs [xr | xr] -> weight blocks (0,0) and (0,C)
    wrx = (drpack(wrt[0:C], [(0, 0), (0, C)]) * WSCALE).astype(NPFP8)
    # pairs: block-diag child weights; pair-4 k-tile 1 = identity (xr inject)
    wrcf = drpack(wrt[C:2 * C], [(0, 0), (C, C)]) * WSCALE
    wrcf[:, 4, 1, :] = WSCALE * np.eye(2 * C)
    wrc = wrcf.astype(NPFP8)

    def wswap(w):  # z/o lhsT with [csum/rh | x] partition order
        a = wt(w)
        return np.ascontiguousarray(
            np.concatenate([a[C:2 * C], a[0:C]], axis=0)
        ).astype(NPBF16)

    wzt, wot = wswap(Wz), wswap(Wo)
    idvt = np.concatenate([np.eye(C), np.eye(C)], axis=0).astype(NPBF16)
    brt = np.tile(np.asarray(br, np.float32).reshape(C, 1), (2, 1))
    bzt = np.asarray(bz, np.float32).reshape(C, 1)
    bot = np.asarray(bo, np.float32).reshape(C, 1)

    in_maps = []
    for k in range(NCORES):
        r0 = k * OUT_ROWS  # slab = global rows r0-2 .. r0+26
        xs = xp[:, r0:r0 + IN_ROWS, :]
        cs = cp[:, :, r0:r0 + IN_ROWS, :]
        in_maps.append({
            "x8t": frame(xs, NPFP8), "xin": frame(xs, NPBF16),
            "cin8": frame(cs, NPFP8), "cin": frame(cs, NPBF16),
            "wrxt": wrx, "wrct": wrc, "wzt": wzt, "wot": wot,
            "idvt": idvt,
            "brt": brt, "bzt": bzt, "bot": bot,
        })
    return in_maps


def run(in_maps, trace=False):
    nc = _get_program()
    return run_bass_kernel_spmd(nc, in_maps, list(range(NCORES)), trace=trace)


def kernel(x, child_h, Wr, br, Wz, bz, Wo, bo):
    in_maps = make_in_maps(x, child_h, Wr, br, Wz, bz, Wo, bo)
    res = run(in_maps).results
    out = np.empty((1, C, HW, HW), dtype=np.float32)
    for k in range(NCORES):
        h = np.asarray(res[k]["hout"]).astype(np.float32)
        h = h.reshape(C, OUT_ROWS, WP)[:, :, 1:1 + HW]
        out[0, :, k * OUT_ROWS:(k + 1) * OUT_ROWS, :] = h
    return out
